# revision 27
# baseline (speedup 1.0000x reference)
"""Causal self-attention Trainium2 kernel (8-core SPMD).

Problem: x[2,2048,1024], causal mask, Wqkv[3072,1024], Wo[1024,1024], fp32.
  qkv = x @ Wqkv.T ; per-head causal softmax attention ; out = attn @ Wo.T

Sharding (data + tensor parallel over heads):
  core c -> batch b = c // 4, heads {4g..4g+3} with g = c % 4.
  Each core computes Q,K,V for its 4 heads, runs causal attention, and
  multiplies by the matching 256 columns of Wo, producing a partial
  [2048, 1024] output (bf16). Host sums the 4 partials per batch.

Perf structure (v2) — engineered around the PE LDWEIGHTS serialization rule
(a stationary load only overlaps an in-flight matmul when their 64-row
groups differ) and ACT exp throughput:
  - All 128-contraction matmuls (qkv proj, AV, out proj) are split into two
    64-row halves with distinct PE row groups: the two half-matmuls execute
    CONCURRENTLY in the array (row tiling) and their weight loads hide under
    the opposite group's stream. Halves land in separate PSUM banks and are
    summed by the DVE on the way to SBUF (replacing the copy the unsplit
    version needed anyway).
  - Scores for a head PAIR (partition bases 0/64 in the qkT layout) run
    concurrently the same way: same k-block, two heads, one 2-bank PSUM
    tile, one 1024-wide exp.
  - AV keeps the ones-column trick per half (stationary [64, 65]): partition
    64 of each half accumulates the softmax denominator; halves+denominators
    are combined during normalization.
  - AV for k-block kb is emitted after scores for kb+2, so PE work per
    ring slot (~0.64us) tracks ACT exp time per slot (~0.8us).
  - Input DMAs are chunked [128,512] and spread over 4 queues so the first
    projection chain starts after ~300KB instead of ~4MB.
"""

import os

import numpy as np

S = 2048
D = 1024
DH = 64
B = 2
NCORES = 8
HPC = 4  # heads per core
QKC = 2 * HPC * DH  # 512 q+k projection columns per core
VC = HPC * DH  # 256 v columns per core
P = 128
H = 64  # row-group half
KO = D // P  # 8 contraction tiles
NQ = S // 512  # 4 q-chunks of 512
NSC = S // P  # 16 s-chunks of 128

COMPUTE_DT = os.environ.get("ATTN_COMPUTE_DT", "bf16")
AVSPLIT = os.environ.get("ATTN_AVSPLIT", "1") == "1"
PAIR = os.environ.get("ATTN_PAIR", "1") == "1"
PROJSPLIT = os.environ.get("ATTN_PROJSPLIT", "1") == "1"
DEBUG = os.environ.get("ATTN_DEBUG", "0") == "1"

_cache = {}


def _np_compute_dt():
    if COMPUTE_DT == "bf16":
        import ml_dtypes

        return ml_dtypes.bfloat16
    return np.float32


def _build():
    import concourse.bacc as bacc
    import concourse.mybir as mybir
    import concourse.tile as tile

    F32 = mybir.dt.float32
    CDT = mybir.dt.bfloat16 if COMPUTE_DT == "bf16" else mybir.dt.float32r
    EXP = mybir.ActivationFunctionType.Exp

    nc = bacc.Bacc()
    xT_d = nc.dram_tensor("xT", [D, S], CDT, kind="ExternalInput")
    wqkT_d = nc.dram_tensor("wqkT", [D, QKC], CDT, kind="ExternalInput")
    wvT_d = nc.dram_tensor("wvT", [D, VC], CDT, kind="ExternalInput")
    woT_d = nc.dram_tensor("woT", [VC, D], CDT, kind="ExternalInput")
    maskT_d = nc.dram_tensor("maskT", [P, P], CDT, kind="ExternalInput")
    out_d = nc.dram_tensor("out", [S, D], CDT, kind="ExternalOutput")
    if DEBUG:
        qkT_dump = nc.dram_tensor("qkT_dump", [P, 4, S], CDT, kind="ExternalOutput")
        v_dump = nc.dram_tensor("v_dump", [P, NSC, HPC, DH + 1], CDT, kind="ExternalOutput")
        attn_dump = nc.dram_tensor("attn_dump", [P, 2, S], CDT, kind="ExternalOutput")

    with tile.TileContext(nc) as tc:
        with (
            tc.tile_pool(name="persist", bufs=1) as persist,
            tc.tile_pool(name="sb_small", bufs=3) as sb_small,
            tc.tile_pool(name="sb_exp", bufs=12) as sb_exp,
            tc.tile_pool(name="sb_out", bufs=3) as sb_out,
            tc.tile_pool(name="pp_big", bufs=2, space="PSUM") as pp_big,
            tc.tile_pool(name="pp_av", bufs=2, space="PSUM") as pp_av,
        ):
            xT_sb = persist.tile([P, KO, S], CDT, tag="xT")
            wqkT_sb = persist.tile([P, KO, QKC], CDT, tag="wqkT")
            wvT_sb = persist.tile([P, KO, VC], CDT, tag="wvT")
            woT_sb = persist.tile([P, 2, D], CDT, tag="woT")
            maskT2_sb = persist.tile([P, 2, P], CDT, tag="maskT2")
            qkT_sb = persist.tile([P, 4, S], CDT, tag="qkT")
            v_sb = persist.tile([P, NSC, HPC, DH + 1], CDT, tag="v")
            attn_sb = persist.tile([P, 2, S], CDT, tag="attn")

            # --- input DMAs: chunked [128,512] and spread over the 3 DMA-
            # capable queues (sync/scalar/gpsimd) so the first proj chains
            # start after ~300KB and qc-chunks land in consumption order.
            def xdma(eng, ko, qc):
                eng.dma_start(
                    xT_sb[:, ko, qc * 512 : (qc + 1) * 512],
                    xT_d[ko * P : (ko + 1) * P, qc * 512 : (qc + 1) * 512],
                )

            for ko in range(KO):
                nc.sync.dma_start(wqkT_sb[:, ko, :], wqkT_d[ko * P : (ko + 1) * P, :])
                xdma(nc.gpsimd, ko, 0)
                nc.sync.dma_start(wvT_sb[:, ko, :], wvT_d[ko * P : (ko + 1) * P, :])
            for ko in range(KO):
                xdma(nc.sync, ko, 1)
                xdma(nc.gpsimd, ko, 2)
                xdma(nc.gpsimd if ko % 2 == 0 else nc.sync, ko, 3)
            for _mh in range(2):
                nc.sync.dma_start(maskT2_sb[:, _mh, :], maskT_d[:])
            nc.gpsimd.dma_start(woT_sb[:], woT_d.rearrange("(ct p) e -> p ct e", p=P))
            ones_c = persist.tile([1, DH], CDT, tag="ones_c")
            nc.vector.memset(ones_c[:], 1.0)
            ones_f32 = persist.tile([P, DH], F32, tag="ones_f32")
            nc.vector.memset(ones_f32[:], 1.0)
            nc.vector.tensor_copy(
                out=v_sb[:, :, :, DH],
                in_=ones_f32[:, 0 : NSC * HPC].rearrange("p (a b) -> p a b", a=NSC),
            )

            def emit_outproj(qc):
                # out[sc*128:(sc+1)*128, :] — both 512-wide en halves chain in
                # one [128,1024] tile; ct pairs share a stationary (attn tile)
                # so consecutive matmuls reuse the loaded weights.
                for si in range(4):
                    sc = qc * 4 + si
                    ps_o = pp_big.tile([P, 1024], F32, tag="big", name="ps_o")
                    for ct in range(2):
                        for en in range(2):
                            nc.tensor.matmul(
                                ps_o[:, en * 512 : (en + 1) * 512],
                                attn_sb[:, ct, sc * P : (sc + 1) * P],
                                woT_sb[:, ct, en * 512 : (en + 1) * 512],
                                start=(ct == 0),
                                stop=(ct == 1),
                                skip_group_check=True,
                            )
                    o_sb = sb_out.tile([P, 1024], CDT, tag="osb")
                    if si % 2 == 0:
                        nc.vector.tensor_copy(out=o_sb[:], in_=ps_o[:])
                    else:
                        nc.scalar.copy(out=o_sb[:], in_=ps_o[:])
                    (nc.sync if si % 2 == 0 else nc.gpsimd).dma_start(
                        out_d[sc * P : (sc + 1) * P, :], o_sb[:]
                    )

            def emit_proj_half(qc, ph):
                # --- qk projection m-tiles {ph, 2+ph} (exactly what attention
                # pair ph of this chunk needs) + v for 2 of the 4 s-chunks.
                for mm in (ph, 2 + ph):
                    pj = pp_big.tile([P, 1024], F32, tag="big", name="pj")
                    if PROJSPLIT:
                        for ko in range(KO):
                            for hf in range(2):
                                nc.tensor.matmul(
                                    pj[:, hf * 512 : hf * 512 + 512],
                                    wqkT_sb[hf * H : hf * H + H, ko, mm * P : (mm + 1) * P],
                                    xT_sb[hf * H : hf * H + H, ko, qc * 512 : (qc + 1) * 512],
                                    start=(ko == 0),
                                    stop=(ko == KO - 1),
                                    skip_group_check=True,
                                )
                        # DVE may read only one PSUM input: stage half B in SBUF.
                        pjB = sb_small.tile([P, 512], F32, tag="pjB")
                        nc.vector.tensor_copy(out=pjB[:], in_=pj[:, 512:1024])
                        nc.vector.tensor_add(
                            out=qkT_sb[:, mm, qc * 512 : (qc + 1) * 512],
                            in0=pj[:, 0:512],
                            in1=pjB[:],
                        )
                    else:
                        for ko in range(KO):
                            nc.tensor.matmul(
                                pj[:, 0:512],
                                wqkT_sb[:, ko, mm * P : (mm + 1) * P],
                                xT_sb[:, ko, qc * 512 : (qc + 1) * 512],
                                start=(ko == 0),
                                stop=(ko == KO - 1),
                                skip_group_check=True,
                            )
                        nc.vector.tensor_copy(
                            out=qkT_sb[:, mm, qc * 512 : (qc + 1) * 512],
                            in_=pj[:, 0:512],
                        )

                # --- v projection for s-chunks 4qc..4qc+3: 2 tiles x 2 sc,
                # each sc a pair of 64-row half-chains (A in bank0, B bank1).
                # One sc per tile: a start=True clears the has_written bits of
                # its whole PSUM bank, so each accumulation chain must own a
                # bank exclusively (half A in bank0, half B in bank1).
                for si in range(2):
                    sc = 4 * qc + 2 * ph + si
                    pv = pp_big.tile([P, 1024], F32, tag="big", name="pv")
                    if PROJSPLIT:
                        for ko in range(KO):
                            for hf in range(2):
                                nc.tensor.matmul(
                                    pv[:, hf * 512 : hf * 512 + VC],
                                    xT_sb[hf * H : hf * H + H, ko, sc * P : (sc + 1) * P],
                                    wvT_sb[hf * H : hf * H + H, ko, :],
                                    start=(ko == 0),
                                    stop=(ko == KO - 1),
                                    skip_group_check=True,
                                )
                        pvB = sb_small.tile([P, VC], F32, tag="pvB")
                        nc.vector.tensor_copy(out=pvB[:], in_=pv[:, 512 : 512 + VC])
                        nc.vector.tensor_add(
                            out=v_sb[:, sc, :, 0:DH],
                            in0=pv[:, 0:VC].rearrange("p (h d) -> p h d", h=HPC),
                            in1=pvB.rearrange("p (h d) -> p h d", h=HPC),
                        )
                    else:
                        for ko in range(KO):
                            nc.tensor.matmul(
                                pv[:, 0:VC],
                                xT_sb[:, ko, sc * P : (sc + 1) * P],
                                wvT_sb[:, ko, :],
                                start=(ko == 0),
                                stop=(ko == KO - 1),
                                skip_group_check=True,
                            )
                        nc.vector.tensor_copy(
                            out=v_sb[:, sc, :, 0:DH],
                            in_=pv[:, 0:VC].rearrange("p (h d) -> p h d", h=HPC),
                        )

            def emit_attn_pair(qc, pr):
                # --- attention for q-chunk qc, head pair (2pr, 2pr+1).
                # Pair heads occupy partition bases 0/64 of the same qkT
                # m-tile, so their score matmuls run concurrently in opposite
                # PE row groups. AV for k-block kb is emitted after scores
                # for kb+2 so PE tracks ACT's exp rate.
                nkb = 4 * qc + 4
                if True:
                    mq = pr  # Q m-tile; K m-tile = 2 + pr
                    ps_avs = [
                        pp_av.tile([DH + 1, 1024], F32, tag="av", name=f"av{hh}")
                        for hh in range(2)
                    ]
                    exps = {}

                    def emit_scores(kb):
                        m = kb - 4 * qc
                        off = max(0, m) * P
                        ps2 = pp_big.tile([P, 1024], F32, tag="big", name="ps2")
                        exp2 = sb_exp.tile([P, 1024], CDT, tag="exp")
                        for hh in range(2):
                            hp = hh * H
                            nc.tensor.matmul(
                                ps2[:, hh * 512 + off : hh * 512 + 512],
                                qkT_sb[hp : hp + H, 2 + mq, kb * P : (kb + 1) * P],
                                qkT_sb[hp : hp + H, mq, qc * 512 + off : (qc + 1) * 512],
                                start=True,
                                stop=True,
                                skip_group_check=True,
                            )
                        if off == 0:
                            nc.scalar.activation(exp2[:], ps2[:], EXP, scale=0.125)
                        else:
                            for hh in range(2):
                                lo = hh * 512 + off
                                nc.scalar.activation(
                                    exp2[:, lo : hh * 512 + 512],
                                    ps2[:, lo : hh * 512 + 512],
                                    EXP,
                                    scale=0.125,
                                )
                        if m >= 0:
                            e2v = exp2.rearrange("p (h q) -> p h q", h=2)[
                                :, :, off : off + P
                            ]
                            nc.vector.tensor_mul(
                                out=e2v, in0=e2v, in1=maskT2_sb[:]
                            )
                        exps[kb] = (exp2, off)

                    def emit_av(kb):
                        exp2, off = exps[kb]
                        for hh in range(2):
                            h = 2 * pr + hh
                            lo = hh * 512 + off
                            if AVSPLIT:
                                for hf in range(2):
                                    nc.tensor.matmul(
                                        ps_avs[hh][:, hf * 512 + off : hf * 512 + 512],
                                        v_sb[hf * H : hf * H + H, kb, h, :],
                                        exp2[hf * H : hf * H + H, lo : (lo - off) + 512],
                                        start=(kb == 0),
                                        stop=(kb == nkb - 1),
                                        skip_group_check=True,
                                    )
                            else:
                                nc.tensor.matmul(
                                    ps_avs[hh][:, off:512],
                                    v_sb[:, kb, h, :],
                                    exp2[:, lo : (lo - off) + 512],
                                    start=(kb == 0),
                                    stop=(kb == nkb - 1),
                                    skip_group_check=True,
                                )

                    for kb in range(nkb):
                        emit_scores(kb)
                        if kb >= 2:
                            emit_av(kb - 2)
                    emit_av(nkb - 2)
                    emit_av(nkb - 1)

                    # normalize (batched per pair): one staged copy of the
                    # B halves (av rows + denominator row), one batched
                    # reciprocal, one full-width multiply for both heads.
                    sums2 = sb_small.tile([1, 1024], F32, tag="sums2")
                    avs2 = []
                    for hh in range(2):
                        ps_av = ps_avs[hh]
                        av_h = sb_small.tile([DH, 512], F32, tag=f"av{hh}")
                        avs2.append(av_h)
                        if AVSPLIT:
                            avB = sb_small.tile([DH + 1, 512], F32, tag="avB")
                            nc.vector.tensor_copy(
                                out=avB[:], in_=ps_av[0 : DH + 1, 512:1024]
                            )
                            nc.vector.tensor_add(
                                out=sums2[:, hh * 512 : (hh + 1) * 512],
                                in0=ps_av[DH : DH + 1, 0:512],
                                in1=avB[DH : DH + 1, :],
                            )
                            nc.vector.tensor_add(
                                out=av_h[:], in0=ps_av[0:DH, 0:512], in1=avB[0:DH, :]
                            )
                        else:
                            nc.vector.tensor_copy(
                                out=sums2[:, hh * 512 : (hh + 1) * 512],
                                in_=ps_av[DH : DH + 1, 0:512],
                            )
                            nc.vector.tensor_copy(
                                out=av_h[:], in_=ps_av[0:DH, 0:512]
                            )
                    recip2 = sb_small.tile([1, 1024], F32, tag="recip2")
                    nc.vector.reciprocal_approx_fast(out=recip2[:], in_=sums2[:])
                    recip_bf = sb_small.tile([1, 1024], CDT, tag="recipbf")
                    nc.vector.tensor_copy(out=recip_bf[:], in_=recip2[:])
                    for hh in range(2):
                        # broadcast 1/sums over the 64 head dims with a K=1
                        # ones-matmul on the PE, landing in the (already
                        # consumed) B half of this head's AV psum tile.
                        bc_ps = ps_avs[hh][0:DH, 512:1024]
                        nc.tensor.matmul(
                            bc_ps,
                            ones_c[:],
                            recip_bf[:, hh * 512 : (hh + 1) * 512],
                            start=True,
                            stop=True,
                            skip_group_check=True,
                        )
                        hp = hh * H
                        nc.vector.tensor_mul(
                            out=attn_sb[hp : hp + DH, pr, qc * 512 : (qc + 1) * 512],
                            in0=avs2[hh][:],
                            in1=bc_ps,
                        )
            # software-pipelined driver: the next chunk's projection half is
            # emitted after each attention pair, so the PE always has ring-
            # independent matmul work queued where the exp/DVE consumers lag.
            emit_proj_half(0, 0)
            emit_proj_half(0, 1)
            for qc in range(NQ):
                for pr in range(2):
                    emit_attn_pair(qc, pr)
                    if qc + 1 < NQ:
                        emit_proj_half(qc + 1, pr)
                if qc > 0:
                    emit_outproj(qc - 1)
            emit_outproj(NQ - 1)
            if DEBUG:
                nc.sync.dma_start(qkT_dump[:], qkT_sb[:])
                nc.sync.dma_start(v_dump[:], v_sb[:])
                nc.sync.dma_start(attn_dump[:], attn_sb[:])

    nc.compile()
    return nc


def _get_nc():
    if "nc" not in _cache:
        _cache["nc"] = _build()
    return _cache["nc"]


def _shard(x, mask, Wqkv, Wo):
    cdt = _np_compute_dt()
    in_maps = []
    # binary mask for the transposed 128x128 diagonal block:
    # valid (mask==0) -> 1.0, masked (-inf/large-negative) -> 0.0
    maskT = np.ascontiguousarray((mask[0, 0, :P, :P].T >= 0).astype(cdt))
    for c in range(NCORES):
        b = c // 4
        g = c % 4
        heads = [4 * g + i for i in range(HPC)]
        q_rows = np.concatenate([np.arange(h * DH, (h + 1) * DH) for h in heads])
        k_rows = D + q_rows
        v_rows = 2 * D + q_rows
        qk_rows = np.concatenate([q_rows, k_rows])
        in_maps.append(
            {
                "xT": np.ascontiguousarray(x[b].T.astype(cdt)),
                "wqkT": np.ascontiguousarray(Wqkv[qk_rows, :].T.astype(cdt)),
                "wvT": np.ascontiguousarray(Wqkv[v_rows, :].T.astype(cdt)),
                "woT": np.ascontiguousarray(Wo[:, q_rows].T.astype(cdt)),
                "maskT": maskT,
            }
        )
    return in_maps


def kernel(x, mask, Wqkv, Wo, _trace=False):
    from concourse.bass_utils import run_bass_kernel_spmd

    x = np.asarray(x, dtype=np.float32)
    mask = np.asarray(mask, dtype=np.float32)
    Wqkv = np.asarray(Wqkv, dtype=np.float32)
    Wo = np.asarray(Wo, dtype=np.float32)

    nc = _get_nc()
    in_maps = _shard(x, mask, Wqkv, Wo)
    res = run_bass_kernel_spmd(nc, in_maps, core_ids=list(range(NCORES)), trace=_trace)
    _cache["last_result"] = res

    out = np.zeros((B, S, D), dtype=np.float32)
    for c in range(NCORES):
        out[c // 4] += res.results[c]["out"].astype(np.float32)
    return out


# revision 28
# speedup vs baseline: 1.0448x; 1.0448x over previous
"""Causal self-attention Trainium2 kernel (8-core SPMD).

Problem: x[2,2048,1024], causal mask, Wqkv[3072,1024], Wo[1024,1024], fp32.
  qkv = x @ Wqkv.T ; per-head causal softmax attention ; out = attn @ Wo.T

Sharding (data + tensor parallel over heads):
  core c -> batch b = c // 4, heads {4g..4g+3} with g = c % 4.
  Each core computes Q,K,V for its 4 heads, runs causal attention, and
  multiplies by the matching 256 columns of Wo, producing a partial
  [2048, 1024] output (bf16). Host sums the 4 partials per batch.

Perf structure (v2) — engineered around the PE LDWEIGHTS serialization rule
(a stationary load only overlaps an in-flight matmul when their 64-row
groups differ) and ACT exp throughput:
  - All 128-contraction matmuls (qkv proj, AV, out proj) are split into two
    64-row halves with distinct PE row groups: the two half-matmuls execute
    CONCURRENTLY in the array (row tiling) and their weight loads hide under
    the opposite group's stream. Halves land in separate PSUM banks and are
    summed by the DVE on the way to SBUF (replacing the copy the unsplit
    version needed anyway).
  - Scores for a head PAIR (partition bases 0/64 in the qkT layout) run
    concurrently the same way: same k-block, two heads, one 2-bank PSUM
    tile, one 1024-wide exp.
  - AV keeps the ones-column trick per half (stationary [64, 65]): partition
    64 of each half accumulates the softmax denominator; halves+denominators
    are combined during normalization.
  - AV for k-block kb is emitted after scores for kb+2, so PE work per
    ring slot (~0.64us) tracks ACT exp time per slot (~0.8us).
  - Input DMAs are chunked [128,512] and spread over 4 queues so the first
    projection chain starts after ~300KB instead of ~4MB.
"""

import os

import numpy as np

S = 2048
D = 1024
DH = 64
B = 2
NCORES = 8
HPC = 4  # heads per core
QKC = 2 * HPC * DH  # 512 q+k projection columns per core
VC = HPC * DH  # 256 v columns per core
P = 128
H = 64  # row-group half
KO = D // P  # 8 contraction tiles
NQ = S // 512  # 4 q-chunks of 512
NSC = S // P  # 16 s-chunks of 128

COMPUTE_DT = os.environ.get("ATTN_COMPUTE_DT", "bf16")
AVSPLIT = os.environ.get("ATTN_AVSPLIT", "1") == "1"
PAIR = os.environ.get("ATTN_PAIR", "1") == "1"
PROJSPLIT = os.environ.get("ATTN_PROJSPLIT", "1") == "1"
DEBUG = os.environ.get("ATTN_DEBUG", "0") == "1"

_cache = {}


def _np_compute_dt():
    if COMPUTE_DT == "bf16":
        import ml_dtypes

        return ml_dtypes.bfloat16
    return np.float32


def _build():
    import concourse.bacc as bacc
    import concourse.mybir as mybir
    import concourse.tile as tile

    F32 = mybir.dt.float32
    CDT = mybir.dt.bfloat16 if COMPUTE_DT == "bf16" else mybir.dt.float32r
    EXP = mybir.ActivationFunctionType.Exp

    nc = bacc.Bacc()
    xT_d = nc.dram_tensor("xT", [D, S], CDT, kind="ExternalInput")
    wqkT_d = nc.dram_tensor("wqkT", [D, QKC], CDT, kind="ExternalInput")
    wvT_d = nc.dram_tensor("wvT", [D, VC], CDT, kind="ExternalInput")
    woT_d = nc.dram_tensor("woT", [VC, D], CDT, kind="ExternalInput")
    maskT_d = nc.dram_tensor("maskT", [P, P], CDT, kind="ExternalInput")
    out_d = nc.dram_tensor("out", [S, D], CDT, kind="ExternalOutput")
    if DEBUG:
        qkT_dump = nc.dram_tensor("qkT_dump", [P, 4, S], CDT, kind="ExternalOutput")
        v_dump = nc.dram_tensor("v_dump", [P, NSC, HPC, DH + 1], CDT, kind="ExternalOutput")
        attn_dump = nc.dram_tensor("attn_dump", [P, 2, S], CDT, kind="ExternalOutput")

    with tile.TileContext(nc) as tc:
        with (
            tc.tile_pool(name="persist", bufs=1) as persist,
            tc.tile_pool(name="sb_small", bufs=3) as sb_small,
            tc.tile_pool(name="sb_exp", bufs=12) as sb_exp,
            tc.tile_pool(name="sb_out", bufs=3) as sb_out,
            tc.tile_pool(name="pp_big", bufs=2, space="PSUM") as pp_big,
            tc.tile_pool(name="pp_av", bufs=2, space="PSUM") as pp_av,
        ):
            xT_sb = persist.tile([P, KO, S], CDT, tag="xT")
            wqkT_sb = persist.tile([P, KO, QKC], CDT, tag="wqkT")
            wvT_sb = persist.tile([P, KO, VC], CDT, tag="wvT")
            woT_sb = persist.tile([P, 2, D], CDT, tag="woT")
            maskT2_sb = persist.tile([P, 2, P], CDT, tag="maskT2")
            qkT_sb = persist.tile([P, 4, S], CDT, tag="qkT")
            v_sb = persist.tile([P, NSC, HPC, DH + 1], CDT, tag="v")
            attn_sb = persist.tile([P, 2, S], CDT, tag="attn")

            # --- input DMAs: chunked [128,512] and spread over the 3 DMA-
            # capable queues (sync/scalar/gpsimd) so the first proj chains
            # start after ~300KB and qc-chunks land in consumption order.
            def xdma(eng, ko, qc):
                eng.dma_start(
                    xT_sb[:, ko, qc * 512 : (qc + 1) * 512],
                    xT_d[ko * P : (ko + 1) * P, qc * 512 : (qc + 1) * 512],
                )

            for ko in range(KO):
                nc.sync.dma_start(wqkT_sb[:, ko, :], wqkT_d[ko * P : (ko + 1) * P, :])
                xdma(nc.gpsimd, ko, 0)
                nc.sync.dma_start(wvT_sb[:, ko, :], wvT_d[ko * P : (ko + 1) * P, :])
            for ko in range(KO):
                xdma(nc.sync, ko, 1)
                xdma(nc.gpsimd, ko, 2)
                xdma(nc.gpsimd if ko % 2 == 0 else nc.sync, ko, 3)
            for _mh in range(2):
                nc.sync.dma_start(maskT2_sb[:, _mh, :], maskT_d[:])
            nc.gpsimd.dma_start(woT_sb[:], woT_d.rearrange("(ct p) e -> p ct e", p=P))
            ones_c = persist.tile([1, DH], CDT, tag="ones_c")
            nc.vector.memset(ones_c[:], 1.0)
            ones_f32 = persist.tile([P, DH], F32, tag="ones_f32")
            nc.vector.memset(ones_f32[:], 1.0)
            nc.vector.tensor_copy(
                out=v_sb[:, :, :, DH],
                in_=ones_f32[:, 0 : NSC * HPC].rearrange("p (a b) -> p a b", a=NSC),
            )

            def emit_outproj(qc, sis=(0, 1, 2, 3)):
                # out[sc*128:(sc+1)*128, :] — both 512-wide en halves chain in
                # one [128,1024] tile; ct pairs share a stationary (attn tile)
                # so consecutive matmuls reuse the loaded weights.
                for si in sis:
                    sc = qc * 4 + si
                    ps_o = pp_big.tile([P, 1024], F32, tag="big", name="ps_o")
                    for ct in range(2):
                        for en in range(2):
                            nc.tensor.matmul(
                                ps_o[:, en * 512 : (en + 1) * 512],
                                attn_sb[:, ct, sc * P : (sc + 1) * P],
                                woT_sb[:, ct, en * 512 : (en + 1) * 512],
                                start=(ct == 0),
                                stop=(ct == 1),
                                skip_group_check=True,
                            )
                    o_sb = sb_out.tile([P, 1024], CDT, tag="osb")
                    if si % 2 == 0:
                        nc.vector.tensor_copy(out=o_sb[:], in_=ps_o[:])
                    else:
                        nc.scalar.copy(out=o_sb[:], in_=ps_o[:])
                    (nc.sync if si % 2 == 0 else nc.gpsimd).dma_start(
                        out_d[sc * P : (sc + 1) * P, :], o_sb[:]
                    )

            def emit_proj_half(qc, ph):
                # --- qk projection m-tiles {ph, 2+ph} (exactly what attention
                # pair ph of this chunk needs) + v for 2 of the 4 s-chunks.
                for mm in (ph, 2 + ph):
                    pj = pp_big.tile([P, 1024], F32, tag="big", name="pj")
                    if PROJSPLIT:
                        for ko in range(KO):
                            for hf in range(2):
                                nc.tensor.matmul(
                                    pj[:, hf * 512 : hf * 512 + 512],
                                    wqkT_sb[hf * H : hf * H + H, ko, mm * P : (mm + 1) * P],
                                    xT_sb[hf * H : hf * H + H, ko, qc * 512 : (qc + 1) * 512],
                                    start=(ko == 0),
                                    stop=(ko == KO - 1),
                                    skip_group_check=True,
                                )
                        # DVE may read only one PSUM input: stage half B in SBUF.
                        pjB = sb_small.tile([P, 512], F32, tag="pjB")
                        nc.vector.tensor_copy(out=pjB[:], in_=pj[:, 512:1024])
                        nc.vector.tensor_add(
                            out=qkT_sb[:, mm, qc * 512 : (qc + 1) * 512],
                            in0=pj[:, 0:512],
                            in1=pjB[:],
                        )
                    else:
                        for ko in range(KO):
                            nc.tensor.matmul(
                                pj[:, 0:512],
                                wqkT_sb[:, ko, mm * P : (mm + 1) * P],
                                xT_sb[:, ko, qc * 512 : (qc + 1) * 512],
                                start=(ko == 0),
                                stop=(ko == KO - 1),
                                skip_group_check=True,
                            )
                        nc.vector.tensor_copy(
                            out=qkT_sb[:, mm, qc * 512 : (qc + 1) * 512],
                            in_=pj[:, 0:512],
                        )

                # --- v projection for s-chunks 4qc..4qc+3: 2 tiles x 2 sc,
                # each sc a pair of 64-row half-chains (A in bank0, B bank1).
                # One sc per tile: a start=True clears the has_written bits of
                # its whole PSUM bank, so each accumulation chain must own a
                # bank exclusively (half A in bank0, half B in bank1).
                for si in range(2):
                    sc = 4 * qc + 2 * ph + si
                    pv = pp_big.tile([P, 1024], F32, tag="big", name="pv")
                    if PROJSPLIT:
                        for ko in range(KO):
                            for hf in range(2):
                                nc.tensor.matmul(
                                    pv[:, hf * 512 : hf * 512 + VC],
                                    xT_sb[hf * H : hf * H + H, ko, sc * P : (sc + 1) * P],
                                    wvT_sb[hf * H : hf * H + H, ko, :],
                                    start=(ko == 0),
                                    stop=(ko == KO - 1),
                                    skip_group_check=True,
                                )
                        pvB = sb_small.tile([P, VC], F32, tag="pvB")
                        nc.vector.tensor_copy(out=pvB[:], in_=pv[:, 512 : 512 + VC])
                        nc.vector.tensor_add(
                            out=v_sb[:, sc, :, 0:DH],
                            in0=pv[:, 0:VC].rearrange("p (h d) -> p h d", h=HPC),
                            in1=pvB.rearrange("p (h d) -> p h d", h=HPC),
                        )
                    else:
                        for ko in range(KO):
                            nc.tensor.matmul(
                                pv[:, 0:VC],
                                xT_sb[:, ko, sc * P : (sc + 1) * P],
                                wvT_sb[:, ko, :],
                                start=(ko == 0),
                                stop=(ko == KO - 1),
                                skip_group_check=True,
                            )
                        nc.vector.tensor_copy(
                            out=v_sb[:, sc, :, 0:DH],
                            in_=pv[:, 0:VC].rearrange("p (h d) -> p h d", h=HPC),
                        )

            def emit_attn_core(qc, pr):
                # --- attention for q-chunk qc, head pair (2pr, 2pr+1).
                # Pair heads occupy partition bases 0/64 of the same qkT
                # m-tile, so their score matmuls run concurrently in opposite
                # PE row groups. AV for k-block kb is emitted after scores
                # for kb+2 so PE tracks ACT's exp rate.
                nkb = 4 * qc + 4
                if True:
                    mq = pr  # Q m-tile; K m-tile = 2 + pr
                    ps_avs = [
                        pp_av.tile([DH + 1, 1024], F32, tag="av", name=f"av{hh}")
                        for hh in range(2)
                    ]
                    exps = {}

                    def emit_scores(kb):
                        m = kb - 4 * qc
                        off = max(0, m) * P
                        ps2 = pp_big.tile([P, 1024], F32, tag="big", name="ps2")
                        exp2 = sb_exp.tile([P, 1024], CDT, tag="exp")
                        for hh in range(2):
                            hp = hh * H
                            nc.tensor.matmul(
                                ps2[:, hh * 512 + off : hh * 512 + 512],
                                qkT_sb[hp : hp + H, 2 + mq, kb * P : (kb + 1) * P],
                                qkT_sb[hp : hp + H, mq, qc * 512 + off : (qc + 1) * 512],
                                start=True,
                                stop=True,
                                skip_group_check=True,
                            )
                        if off == 0:
                            nc.scalar.activation(exp2[:], ps2[:], EXP, scale=0.125)
                        else:
                            for hh in range(2):
                                lo = hh * 512 + off
                                nc.scalar.activation(
                                    exp2[:, lo : hh * 512 + 512],
                                    ps2[:, lo : hh * 512 + 512],
                                    EXP,
                                    scale=0.125,
                                )
                        if m >= 0:
                            e2v = exp2.rearrange("p (h q) -> p h q", h=2)[
                                :, :, off : off + P
                            ]
                            nc.vector.tensor_mul(
                                out=e2v, in0=e2v, in1=maskT2_sb[:]
                            )
                        exps[kb] = (exp2, off)

                    def emit_av(kb):
                        exp2, off = exps[kb]
                        for hh in range(2):
                            h = 2 * pr + hh
                            lo = hh * 512 + off
                            if AVSPLIT:
                                for hf in range(2):
                                    nc.tensor.matmul(
                                        ps_avs[hh][:, hf * 512 + off : hf * 512 + 512],
                                        v_sb[hf * H : hf * H + H, kb, h, :],
                                        exp2[hf * H : hf * H + H, lo : (lo - off) + 512],
                                        start=(kb == 0),
                                        stop=(kb == nkb - 1),
                                        skip_group_check=True,
                                    )
                            else:
                                nc.tensor.matmul(
                                    ps_avs[hh][:, off:512],
                                    v_sb[:, kb, h, :],
                                    exp2[:, lo : (lo - off) + 512],
                                    start=(kb == 0),
                                    stop=(kb == nkb - 1),
                                    skip_group_check=True,
                                )

                    for kb in range(nkb):
                        emit_scores(kb)
                        if kb >= 2:
                            emit_av(kb - 2)
                    emit_av(nkb - 2)
                    emit_av(nkb - 1)
                    return ps_avs

            def emit_normalize(qc, pr, ps_avs):
                if True:
                    # normalize (batched per pair): one staged copy of the
                    # B halves (av rows + denominator row), one batched
                    # reciprocal, one full-width multiply for both heads.
                    sums2 = sb_small.tile([1, 1024], F32, tag="sums2")
                    avs2 = []
                    for hh in range(2):
                        ps_av = ps_avs[hh]
                        av_h = sb_small.tile([DH, 512], F32, tag=f"av{hh}")
                        avs2.append(av_h)
                        if AVSPLIT:
                            avB = sb_small.tile([DH + 1, 512], F32, tag="avB")
                            nc.vector.tensor_copy(
                                out=avB[:], in_=ps_av[0 : DH + 1, 512:1024]
                            )
                            nc.vector.tensor_add(
                                out=sums2[:, hh * 512 : (hh + 1) * 512],
                                in0=ps_av[DH : DH + 1, 0:512],
                                in1=avB[DH : DH + 1, :],
                            )
                            nc.vector.tensor_add(
                                out=av_h[:], in0=ps_av[0:DH, 0:512], in1=avB[0:DH, :]
                            )
                        else:
                            nc.vector.tensor_copy(
                                out=sums2[:, hh * 512 : (hh + 1) * 512],
                                in_=ps_av[DH : DH + 1, 0:512],
                            )
                            nc.vector.tensor_copy(
                                out=av_h[:], in_=ps_av[0:DH, 0:512]
                            )
                    recip2 = sb_small.tile([1, 1024], F32, tag="recip2")
                    nc.vector.reciprocal_approx_fast(out=recip2[:], in_=sums2[:])
                    recip_bf = sb_small.tile([1, 1024], CDT, tag="recipbf")
                    nc.vector.tensor_copy(out=recip_bf[:], in_=recip2[:])
                    for hh in range(2):
                        # broadcast 1/sums over the 64 head dims with a K=1
                        # ones-matmul on the PE, landing in the (already
                        # consumed) B half of this head's AV psum tile.
                        bc_ps = ps_avs[hh][0:DH, 512:1024]
                        nc.tensor.matmul(
                            bc_ps,
                            ones_c[:],
                            recip_bf[:, hh * 512 : (hh + 1) * 512],
                            start=True,
                            stop=True,
                            skip_group_check=True,
                        )
                        hp = hh * H
                        nc.vector.tensor_mul(
                            out=attn_sb[hp : hp + DH, pr, qc * 512 : (qc + 1) * 512],
                            in0=avs2[hh][:],
                            in1=bc_ps,
                        )
            # software-pipelined driver: after each attention pair's matmuls,
            # ~10us of ring-independent projection (or out-projection) matmuls
            # are queued BEFORE the pair's normalize, so the PE never idles at
            # the pair boundary while ACT/DVE drain the exp tail, and HAM
            # stays warm.
            emit_proj_half(0, 0)
            emit_proj_half(0, 1)
            for qc in range(NQ):
                for pr in range(2):
                    ps_avs = emit_attn_core(qc, pr)
                    if qc + 1 < NQ:
                        emit_proj_half(qc + 1, pr)
                    elif qc > 0:
                        emit_outproj(qc - 1, (2 * pr, 2 * pr + 1))
                    emit_normalize(qc, pr, ps_avs)
                if 0 < qc < NQ - 1:
                    emit_outproj(qc - 1)
            emit_outproj(NQ - 1)
            if DEBUG:
                nc.sync.dma_start(qkT_dump[:], qkT_sb[:])
                nc.sync.dma_start(v_dump[:], v_sb[:])
                nc.sync.dma_start(attn_dump[:], attn_sb[:])

    nc.compile()
    return nc


def _get_nc():
    if "nc" not in _cache:
        _cache["nc"] = _build()
    return _cache["nc"]


def _shard(x, mask, Wqkv, Wo):
    cdt = _np_compute_dt()
    in_maps = []
    # binary mask for the transposed 128x128 diagonal block:
    # valid (mask==0) -> 1.0, masked (-inf/large-negative) -> 0.0
    maskT = np.ascontiguousarray((mask[0, 0, :P, :P].T >= 0).astype(cdt))
    for c in range(NCORES):
        b = c // 4
        g = c % 4
        heads = [4 * g + i for i in range(HPC)]
        q_rows = np.concatenate([np.arange(h * DH, (h + 1) * DH) for h in heads])
        k_rows = D + q_rows
        v_rows = 2 * D + q_rows
        qk_rows = np.concatenate([q_rows, k_rows])
        in_maps.append(
            {
                "xT": np.ascontiguousarray(x[b].T.astype(cdt)),
                "wqkT": np.ascontiguousarray(Wqkv[qk_rows, :].T.astype(cdt)),
                "wvT": np.ascontiguousarray(Wqkv[v_rows, :].T.astype(cdt)),
                "woT": np.ascontiguousarray(Wo[:, q_rows].T.astype(cdt)),
                "maskT": maskT,
            }
        )
    return in_maps


def kernel(x, mask, Wqkv, Wo, _trace=False):
    from concourse.bass_utils import run_bass_kernel_spmd

    x = np.asarray(x, dtype=np.float32)
    mask = np.asarray(mask, dtype=np.float32)
    Wqkv = np.asarray(Wqkv, dtype=np.float32)
    Wo = np.asarray(Wo, dtype=np.float32)

    nc = _get_nc()
    in_maps = _shard(x, mask, Wqkv, Wo)
    res = run_bass_kernel_spmd(nc, in_maps, core_ids=list(range(NCORES)), trace=_trace)
    _cache["last_result"] = res

    out = np.zeros((B, S, D), dtype=np.float32)
    for c in range(NCORES):
        out[c // 4] += res.results[c]["out"].astype(np.float32)
    return out


# revision 29
# speedup vs baseline: 1.2745x; 1.2199x over previous
"""Causal self-attention Trainium2 kernel (8-core SPMD).

Problem: x[2,2048,1024], causal mask, Wqkv[3072,1024], Wo[1024,1024], fp32.
  qkv = x @ Wqkv.T ; per-head causal softmax attention ; out = attn @ Wo.T

Sharding (data + tensor parallel over heads):
  core c -> batch b = c // 4, heads {4g..4g+3} with g = c % 4.
  Each core computes Q,K,V for its 4 heads, runs causal attention for them,
  and multiplies by the matching 256 columns of Wo, producing a partial
  [2048, 1024] bf16 output. Host sums the 4 partials per batch in fp32.

Kernel structure (per core): baseline dense-emission skeleton with
  - bf16 matmul operands, fp32 PSUM accumulation.
  - PAIRED scores: heads 2p/2p+1 live at partition bases 0/64 of the same
    qkT m-tile, so their K^T Q matmuls (64-row stationaries) execute
    CONCURRENTLY in opposite PE row groups (row tiling), writing the two
    banks of one [128,1024] PSUM tile; one 1024-wide exp covers both.
  - AV per head unsplit ([128,65] stationary with a ones column accumulating
    the softmax denominator), emitted 2 k-blocks behind the paired scores so
    the PE tracks ACT's exp rate; normalization reads PSUM directly.
  - Causality: strictly-upper blocks skipped; diagonal straddlers compute
    only valid columns; the 128-wide diagonal block is exp'd unmasked and
    both heads are masked with one strided multiply against a stacked
    binary mask tile.
  - Input DMAs chunked [128,512] over the 3 DMA queues in consumption
    order, so the first projection chain starts after ~300KB.
"""

import os

import numpy as np

S = 2048
D = 1024
DH = 64
B = 2
NCORES = 8
HPC = 4  # heads per core
QKC = 2 * HPC * DH  # 512 q+k projection columns per core
VC = HPC * DH  # 256 v columns per core
P = 128
H = 64
KO = D // P  # 8 contraction tiles
NQ = S // 512  # 4 q-chunks of 512
NSC = S // P  # 16 s-chunks of 128

COMPUTE_DT = os.environ.get("ATTN_COMPUTE_DT", "bf16")
DEBUG = os.environ.get("ATTN_DEBUG", "0") == "1"

_cache = {}


def _np_compute_dt():
    if COMPUTE_DT == "bf16":
        import ml_dtypes

        return ml_dtypes.bfloat16
    return np.float32


def _build():
    import concourse.bacc as bacc
    import concourse.mybir as mybir
    import concourse.tile as tile

    F32 = mybir.dt.float32
    CDT = mybir.dt.bfloat16 if COMPUTE_DT == "bf16" else mybir.dt.float32r
    EXP = mybir.ActivationFunctionType.Exp

    nc = bacc.Bacc()
    xT_d = nc.dram_tensor("xT", [D, S], CDT, kind="ExternalInput")
    wqkT_d = nc.dram_tensor("wqkT", [D, QKC], CDT, kind="ExternalInput")
    wvT_d = nc.dram_tensor("wvT", [D, VC], CDT, kind="ExternalInput")
    woT_d = nc.dram_tensor("woT", [VC, D], CDT, kind="ExternalInput")
    maskT_d = nc.dram_tensor("maskT", [P, P], CDT, kind="ExternalInput")
    out_d = nc.dram_tensor("out", [S, D], CDT, kind="ExternalOutput")
    if DEBUG:
        qkT_dump = nc.dram_tensor("qkT_dump", [P, 4, S], CDT, kind="ExternalOutput")
        v_dump = nc.dram_tensor("v_dump", [P, NSC, HPC, DH + 1], CDT, kind="ExternalOutput")
        attn_dump = nc.dram_tensor("attn_dump", [P, 2, S], CDT, kind="ExternalOutput")

    with tile.TileContext(nc) as tc:
        with (
            tc.tile_pool(name="persist", bufs=1) as persist,
            tc.tile_pool(name="sb_small", bufs=3) as sb_small,
            tc.tile_pool(name="sb_exp", bufs=12) as sb_exp,
            tc.tile_pool(name="sb_out", bufs=3) as sb_out,
            tc.tile_pool(name="pp_big", bufs=2, space="PSUM") as pp_big,
            tc.tile_pool(name="pp_av", bufs=2, space="PSUM") as pp_av,
            tc.tile_pool(name="pp_o", bufs=2, space="PSUM") as pp_o,
        ):
            xT_sb = persist.tile([P, KO, S], CDT, tag="xT")
            wqkT_sb = persist.tile([P, KO, QKC], CDT, tag="wqkT")
            wvT_sb = persist.tile([P, KO, VC], CDT, tag="wvT")
            woT_sb = persist.tile([P, 2, D], CDT, tag="woT")
            maskT2_sb = persist.tile([P, 2, P], CDT, tag="maskT2")
            qkT_sb = persist.tile([P, 4, S], CDT, tag="qkT")
            v_sb = persist.tile([P, NSC, HPC, DH + 1], CDT, tag="v")
            attn_sb = persist.tile([P, 2, S], CDT, tag="attn")

            def xdma(eng, ko, qc):
                eng.dma_start(
                    xT_sb[:, ko, qc * 512 : (qc + 1) * 512],
                    xT_d[ko * P : (ko + 1) * P, qc * 512 : (qc + 1) * 512],
                )

            for ko in range(KO):
                nc.sync.dma_start(wqkT_sb[:, ko, :], wqkT_d[ko * P : (ko + 1) * P, :])
                xdma(nc.gpsimd, ko, 0)
                nc.scalar.dma_start(wvT_sb[:, ko, :], wvT_d[ko * P : (ko + 1) * P, :])
            for ko in range(KO):
                xdma(nc.sync, ko, 1)
                xdma(nc.gpsimd if ko % 2 == 0 else nc.scalar, ko, 2)
                xdma(nc.scalar if ko % 2 == 0 else nc.gpsimd, ko, 3)
            for _mh in range(2):
                nc.sync.dma_start(maskT2_sb[:, _mh, :], maskT_d[:])
            nc.scalar.dma_start(woT_sb[:], woT_d.rearrange("(ct p) e -> p ct e", p=P))
            ones_f32 = persist.tile([P, DH], F32, tag="ones_f32")
            nc.vector.memset(ones_f32[:], 1.0)
            nc.vector.tensor_copy(
                out=v_sb[:, :, :, DH],
                in_=ones_f32[:, 0 : NSC * HPC].rearrange("p (a b) -> p a b", a=NSC),
            )

            def emit_outproj(qc):
                for si in range(4):
                    sc = qc * 4 + si
                    for en in range(2):
                        ps_o = pp_o.tile([P, 512], F32, tag="o")
                        for ct in range(2):
                            nc.tensor.matmul(
                                ps_o[:],
                                attn_sb[:, ct, sc * P : (sc + 1) * P],
                                woT_sb[:, ct, en * 512 : (en + 1) * 512],
                                start=(ct == 0),
                                stop=(ct == 1),
                                skip_group_check=True,
                            )
                        o_sb = sb_out.tile([P, 512], CDT, tag="osb")
                        nc.vector.tensor_copy(out=o_sb[:], in_=ps_o[:])
                        (nc.sync if (si + en) % 2 == 0 else nc.gpsimd).dma_start(
                            out_d[sc * P : (sc + 1) * P, en * 512 : (en + 1) * 512],
                            o_sb[:],
                        )

            for qc in range(NQ):
                # --- qk projection: ko-outer over two 2-bank tiles (4
                # half-bank chains) so the PE tracks DMA arrival ---
                pjA = pp_big.tile([P, 1024], F32, tag="big", name="pjA")
                pjB = pp_big.tile([P, 1024], F32, tag="big", name="pjB")
                for ko in range(KO):
                    for mm in range(4):
                        slot = pjA if mm < 2 else pjB
                        nc.tensor.matmul(
                            slot[:, (mm % 2) * 512 : (mm % 2 + 1) * 512],
                            wqkT_sb[:, ko, mm * P : (mm + 1) * P],
                            xT_sb[:, ko, qc * 512 : (qc + 1) * 512],
                            start=(ko == 0),
                            stop=(ko == KO - 1),
                            skip_group_check=True,
                        )
                nc.vector.tensor_copy(
                    out=qkT_sb[:, 0:2, qc * 512 : (qc + 1) * 512],
                    in_=pjA.rearrange("p (a b) -> p a b", a=2),
                )
                nc.vector.tensor_copy(
                    out=qkT_sb[:, 2:4, qc * 512 : (qc + 1) * 512],
                    in_=pjB.rearrange("p (a b) -> p a b", a=2),
                )

                # --- v projection for s-chunks 4qc..4qc+3 (4 bank chains) ---
                pvA = pp_big.tile([P, 1024], F32, tag="big", name="pvA")
                pvB = pp_big.tile([P, 1024], F32, tag="big", name="pvB")
                for ko in range(KO):
                    for j in range(4):
                        slot = pvA if j < 2 else pvB
                        sc = 4 * qc + j
                        nc.tensor.matmul(
                            slot[:, (j % 2) * 512 : (j % 2) * 512 + VC],
                            xT_sb[:, ko, sc * P : (sc + 1) * P],
                            wvT_sb[:, ko, :],
                            start=(ko == 0),
                            stop=(ko == KO - 1),
                            skip_group_check=True,
                        )
                for half, slot in ((0, pvA), (1, pvB)):
                    nc.vector.tensor_copy(
                        out=v_sb[:, 4 * qc + 2 * half : 4 * qc + 2 * half + 2, :, 0:DH],
                        in_=slot.rearrange("p (a h d) -> p a h d", a=2, h=8)[:, :, 0:HPC, :],
                    )

                # --- attention for q-chunk qc, head pairs (0,1), (2,3).
                # Pair heads sit at partition bases 0/64 of one qkT m-tile,
                # so the two 64-row score matmuls of a k-block execute
                # concurrently in opposite PE row groups (one 2-bank tile,
                # one 1024-wide exp). AV (unsplit, ones-column denominator)
                # trails the scores by 2 k-blocks.
                nkb = 4 * qc + 4
                for pr in range(2):
                    mq = pr  # Q m-tile; K m-tile = 2 + pr
                    ps_avs = [
                        pp_av.tile([DH + 1, 512], F32, tag="av", name=f"av{hh}")
                        for hh in range(2)
                    ]
                    exps = {}

                    def emit_scores(kb):
                        m = kb - 4 * qc
                        off = max(0, m) * P
                        ps2 = pp_big.tile([P, 1024], F32, tag="big", name="ps2")
                        exp2 = sb_exp.tile([P, 1024], CDT, tag="exp")
                        for hh in range(2):
                            hp = hh * H
                            nc.tensor.matmul(
                                ps2[:, hh * 512 + off : hh * 512 + 512],
                                qkT_sb[hp : hp + H, 2 + mq, kb * P : (kb + 1) * P],
                                qkT_sb[hp : hp + H, mq, qc * 512 + off : (qc + 1) * 512],
                                start=True,
                                stop=True,
                                skip_group_check=True,
                            )
                        if off == 0:
                            nc.scalar.activation(exp2[:], ps2[:], EXP, scale=0.125)
                        else:
                            for hh in range(2):
                                lo = hh * 512 + off
                                nc.scalar.activation(
                                    exp2[:, lo : hh * 512 + 512],
                                    ps2[:, lo : hh * 512 + 512],
                                    EXP,
                                    scale=0.125,
                                )
                        if m >= 0:
                            e2v = exp2.rearrange("p (h q) -> p h q", h=2)[
                                :, :, off : off + P
                            ]
                            nc.vector.tensor_mul(out=e2v, in0=e2v, in1=maskT2_sb[:])
                        exps[kb] = (exp2, off)

                    def emit_av(kb):
                        exp2, off = exps[kb]
                        for hh in range(2):
                            h = 2 * pr + hh
                            lo = hh * 512 + off
                            nc.tensor.matmul(
                                ps_avs[hh][:, off:512],
                                v_sb[:, kb, h, :],
                                exp2[:, lo : (lo - off) + 512],
                                start=(kb == 0),
                                stop=(kb == nkb - 1),
                                skip_group_check=True,
                            )

                    for kb in range(nkb):
                        emit_scores(kb)
                        if kb >= 2:
                            emit_av(kb - 2)
                    emit_av(nkb - 2)
                    emit_av(nkb - 1)

                    # normalize: attn = av * (1/sums), reciprocal broadcast
                    # over the 64 head dims via GPSIMD (keeps the PE out of
                    # the pair-boundary dependency chain).
                    sums2 = sb_small.tile([1, 1024], F32, tag="sums2")
                    for hh in range(2):
                        nc.vector.tensor_copy(
                            out=sums2[:, hh * 512 : (hh + 1) * 512],
                            in_=ps_avs[hh][DH : DH + 1, :],
                        )
                    recip2 = sb_small.tile([1, 1024], F32, tag="recip2")
                    nc.vector.reciprocal_approx_fast(out=recip2[:], in_=sums2[:])
                    for hh in range(2):
                        bc_sb = sb_small.tile([DH, 512], F32, tag=f"bc{hh}")
                        nc.gpsimd.partition_broadcast(
                            bc_sb[:], recip2[:, hh * 512 : (hh + 1) * 512]
                        )
                        hp = hh * H
                        nc.vector.tensor_mul(
                            out=attn_sb[hp : hp + DH, pr, qc * 512 : (qc + 1) * 512],
                            in0=ps_avs[hh][0:DH, :],
                            in1=bc_sb[:],
                        )

                # --- deferred output projection (previous q chunk) ---
                if qc > 0:
                    emit_outproj(qc - 1)
            emit_outproj(NQ - 1)
            if DEBUG:
                nc.sync.dma_start(qkT_dump[:], qkT_sb[:])
                nc.sync.dma_start(v_dump[:], v_sb[:])
                nc.sync.dma_start(attn_dump[:], attn_sb[:])

    nc.compile()
    return nc


def _get_nc():
    if "nc" not in _cache:
        _cache["nc"] = _build()
    return _cache["nc"]


def _shard(x, mask, Wqkv, Wo):
    cdt = _np_compute_dt()
    in_maps = []
    # binary mask for the transposed 128x128 diagonal block:
    # valid (mask==0) -> 1.0, masked (-inf/large-negative) -> 0.0
    maskT = np.ascontiguousarray((mask[0, 0, :P, :P].T >= 0).astype(cdt))
    for c in range(NCORES):
        b = c // 4
        g = c % 4
        heads = [4 * g + i for i in range(HPC)]
        q_rows = np.concatenate([np.arange(h * DH, (h + 1) * DH) for h in heads])
        k_rows = D + q_rows
        v_rows = 2 * D + q_rows
        qk_rows = np.concatenate([q_rows, k_rows])
        in_maps.append(
            {
                "xT": np.ascontiguousarray(x[b].T.astype(cdt)),
                "wqkT": np.ascontiguousarray(Wqkv[qk_rows, :].T.astype(cdt)),
                "wvT": np.ascontiguousarray(Wqkv[v_rows, :].T.astype(cdt)),
                "woT": np.ascontiguousarray(Wo[:, q_rows].T.astype(cdt)),
                "maskT": maskT,
            }
        )
    return in_maps


def kernel(x, mask, Wqkv, Wo, _trace=False):
    from concourse.bass_utils import run_bass_kernel_spmd

    x = np.asarray(x, dtype=np.float32)
    mask = np.asarray(mask, dtype=np.float32)
    Wqkv = np.asarray(Wqkv, dtype=np.float32)
    Wo = np.asarray(Wo, dtype=np.float32)

    nc = _get_nc()
    in_maps = _shard(x, mask, Wqkv, Wo)
    res = run_bass_kernel_spmd(nc, in_maps, core_ids=list(range(NCORES)), trace=_trace)
    _cache["last_result"] = res

    out = np.zeros((B, S, D), dtype=np.float32)
    for c in range(NCORES):
        out[c // 4] += res.results[c]["out"].astype(np.float32)
    return out
